# revision 35
# baseline (speedup 1.0000x reference)
"""4-layer GAT + BN + ReLU + linear head on 8 Trainium2 NeuronCores.

Self-contained: takes FULL inputs (as produced by the problem's setup_inputs),
returns the FULL [50000, 32] float32 output.

Strategy (memory-regime, node-sharded):
- Nodes sharded contiguously across 8 cores (6250 each). Per layer:
    M:  h_ext_shard = zT_shard @ W_ext (W_ext = [W_perm | W@A_s | W@A_d], f32);
        al_d kept f32 in per-tile aldD tables (256B gather rows). M of layer
        l+1 is FUSED into layer l's BN loop so the AllGather fires sooner.
    AG: 2 AllGathers over a 40/60 row split (piece-0 small enough that each
        dst tile's gather group fits one 1024-idx q7 call).
    E:  per 128-dst-node tile, dst-sorted edges: dma_gather fetches h_ext[src]
        rows; al_d[dst] via 256B-row gathers over the tile's own aldD (issued
        before the h gathers / prefetched ahead of the AG - no AG dependency).
        Feature columns are head-INTERLEAVED (col = c*4+h) so the per-edge
        ex*h scaling is ONE per-tile 4D DVE op with a packed 4-wide last dim
        (2x 16-bit mode on f16 layers). exp(leaky_relu) = max(exp(x),
        exp(0.2x)) on the Act engine (single act-table set - no reloads),
        tv clamped to [-15, 8] (inactive in practice, f16-overflow guard).
        One-hot (iota==dstlocal) scatter matmuls accumulate y in PSUM;
        normalization after aggregation: y = (sum ex*h)/(sum ex).
    R:  BN statistics (col sums/sumsq via ones-matmuls) AllReduced (2KB).
    BN: y tiles reloaded f32, transposed (PE), relu(S*y+B) fused into one
        Act op per half writing zT; quantization only AFTER the BN affine.
- PRECISION (the 2e-2 gate needs care - per-layer noise amplifies ~2.7x per
  layer through the attention-average + BN-rescale): layers 1-2 run fully
  f32 (rows 1280B, f32 scatter); layers 3-4 use f16 rows (768B, with al_s
  carried as a bit-cast f32 pair inside the f16 row) and f16 scatter
  matmuls (1 cyc/row). zT / W / M matmuls are always f32: measured rel err
  0.0109 vs 0.028+ for any config with f16 anywhere in layers 1-2.
- GATConv bias b is dropped: BN(y + b) == BN(y) exactly.
- Gather tables are laid out in AllGather output order (rank-major pieces),
  so int16 indices always fit. idx + al_d idx tables are merged into one
  combined load per tile.
- SPMD: one NEFF for all 8 cores. All per-core variation lives in input
  tensors; per-tile group sizes are padded to the max over cores.
"""

import os

import numpy as np

import concourse.bacc as bacc
import concourse.mybir as mybir
import concourse.tile as tile
from concourse import bass_utils
from concourse.masks import make_identity

dt = mybir.dt
ALU = mybir.AluOpType
AFT = mybir.ActivationFunctionType

N, E, H, C = 50000, 800000, 4, 64
NCORES, SHARD, P = 8, 6250, 128
NTILE = (SHARD + P - 1) // P  # 49
H0 = 2500  # AG piece sizes: 40/60 split keeps tile group-0 under the 1024-idx
H1 = SHARD - H0  # gather cap (1 call) while group-1 takes 2 -> 3 calls/tile
HB = (0, H0, SHARD)  # piece boundaries
F = H * C  # 256
EPS = 1e-5
WCOL = 264  # W_ext width: 256 h + 4 als + 4 ald
GCAP = int(os.environ.get("GAT_GCAP", "1024"))
SCRATCH = 16384 if GCAP <= 1024 else (24576 if GCAP <= 1536 else 32768)

# precision knobs (debug/bisect): f16 is the perf config
_DTS = {"f16": np.float16, "f32": np.float32, "bf16": None}
ROW_DT = os.environ.get("GAT_ROWDT", "f16")  # gathered h_ext rows (l>=2)
G2_DT = os.environ.get("GAT_G2DT", "f16")  # ex*h + onehot (l>=2)
ZT_DT = os.environ.get("GAT_ZTDT", "f32")  # zT / W / M matmul (l>=2)
NF32 = int(os.environ.get("GAT_NF32", "2"))  # first NF32 layers fully f32
NF32G2 = int(os.environ.get("GAT_NF32G2", "1"))  # G2/scatter f32 through layer NF32G2
NF32ROW = int(os.environ.get("GAT_NF32ROW", "0"))  # rows f32 through layer NF32ROW


def _row_elems(name):
    return 384 if name == "f16" else 320  # 768B f16 / 1280B f32 gather rows


def _lrow_dt(l):
    return "f32" if l <= max(NF32, NF32ROW) else ROW_DT


ROWMAX = max(_row_elems(_lrow_dt(l)) for l in range(1, 5))

# interleaved head layout: new col j = c*4 + h  <->  old col h*64 + c
PERM = np.array([(j % 4) * 64 + j // 4 for j in range(F)], dtype=np.int64)

LAST_RESULTS = None
_CACHE: dict = {}


def _cnt(t):
    return min(P, SHARD - t * P)


# --------------------------------------------------------------------------
# host-side graph preprocessing
# --------------------------------------------------------------------------

def _prep_graph(ei):
    """Per-core packed gather-index / dstlocal arrays + shared meta.

    Edge sources are split per (dst-)tile into AG-half groups; indices are
    positions in the half-table: half h row = rank*HALF + (src%SHARD - h*HALF).
    Returns (idx_arrs [128, ICOLS] i16, idxD_arrs [128, CCOLS*8] i16,
    dfl_arrs [128, CCOLS] f32, meta, ICOLS, CCOLS).
    """
    src = np.asarray(ei[0], np.int64)
    dst = np.asarray(ei[1], np.int64)
    sl = np.arange(N, dtype=np.int64)
    src = np.concatenate([src, sl])
    dst = np.concatenate([dst, sl])
    order = np.argsort(dst, kind="stable")
    src, dst = src[order], dst[order]

    bounds = [
        min(r * SHARD + t * P, (r + 1) * SHARD)
        for r in range(NCORES)
        for t in range(NTILE)
    ] + [N]
    starts = np.searchsorted(dst, np.asarray(bounds))

    s_rank = src // SHARD
    s_loc = src % SHARD
    s_half = (s_loc >= H0).astype(np.int64)
    piece = np.where(s_half == 0, H0, H1)
    s_idx = s_rank * piece + s_loc - s_half * H0  # row in piece-table

    raw = [[None] * NTILE for _ in range(NCORES)]
    for r in range(NCORES):
        for t in range(NTILE):
            g = r * NTILE + t
            a, b = starts[g], starts[g + 1]
            dl = dst[a:b] - (r * SHARD + t * P)
            hh = s_half[a:b]
            ii = s_idx[a:b]
            m0 = hh == 0
            raw[r][t] = ((ii[m0], dl[m0]), (ii[~m0], dl[~m0]))

    npad = np.zeros((NTILE, 2), np.int64)
    for t in range(NTILE):
        for g in range(2):
            mx = max(len(raw[r][t][g][0]) for r in range(NCORES))
            npad[t, g] = ((max(mx, 1) + P - 1) // P) * P

    meta = []
    icol = ccol = 0
    for t in range(NTILE):
        groups = []
        cbase = 0
        for g in range(2):
            n = int(npad[t, g])
            nchunk = n // P
            groups.append((icol, n, g, cbase, nchunk))
            icol += n // 16
            cbase += nchunk
        ccol += cbase
        meta.append(groups)
    ICOLS, CCOLS = icol, ccol

    idx_arrs, idxD_arrs, dfl_arrs = [], [], []
    for r in range(NCORES):
        idx = np.zeros((16, ICOLS), np.int16)
        idxD = np.zeros((16, CCOLS * 8), np.int16)
        dfl = np.full((P, CCOLS), 200.0, np.float32)
        ccur = 0
        for t in range(NTILE):
            for g in range(2):
                ic, n, _, cb, nchunk = meta[t][g]
                s_t, dl = raw[r][t][g]
                ii = np.zeros(n, np.int16)
                ii[: len(s_t)] = s_t.astype(np.int16)
                idx[:, ic : ic + n // 16] = ii.reshape(n // 16, 16).T
                dd = np.full(n, 200.0, np.float32)
                dd[: len(dl)] = dl.astype(np.float32)
                dfl[:, ccur + cb : ccur + cb + nchunk] = dd.reshape(nchunk, P).T
                di = np.zeros(n, np.int16)
                di[: len(dl)] = dl.astype(np.int16)
                dc = (ccur + cb) * 8
                idxD[:, dc : dc + n // 16] = di.reshape(n // 16, 16).T
            ccur += meta[t][0][4] + meta[t][1][4]
        # combined per-tile layout: [tile h-idx cols | tile iD cols]
        comb = np.zeros((16, ICOLS + CCOLS * 8), np.int16)
        ccur = 0
        for t in range(NTILE):
            ic0 = meta[t][0][0]
            itcols = (meta[t][0][1] + meta[t][1][1]) // 16
            S_t = meta[t][0][4] + meta[t][1][4]
            o = ic0 + ccur * 8
            comb[:, o : o + itcols] = idx[:, ic0 : ic0 + itcols]
            comb[:, o + itcols : o + itcols + S_t * 8] = idxD[
                :, ccur * 8 : (ccur + S_t) * 8
            ]
            ccur += S_t
        idx_arrs.append(np.tile(comb, (8, 1)).astype(np.int16))
        dfl_arrs.append(dfl)
    return idx_arrs, dfl_arrs, meta, ICOLS, CCOLS


def _make_wext(W, a_s, a_d, permute_rows, out_dt):
    """[kin, WCOL] f16: cols 0:256 = W col-permuted (interleave), 256:260 =
    W@A_s, 260:264 = W@A_d. Rows permuted for layers whose input is zT
    (feature rows in interleaved order)."""
    ind = W.shape[0]
    A_s = np.zeros((F, H), np.float32)
    A_d = np.zeros((F, H), np.float32)
    for h in range(H):
        A_s[h * C : (h + 1) * C, h] = a_s[h]
        A_d[h * C : (h + 1) * C, h] = a_d[h]
    Wx = np.zeros((ind, WCOL), np.float32)
    Wx[:, :F] = W[:, PERM]
    Wx[:, 256:260] = W @ A_s
    Wx[:, 260:264] = W @ A_d
    if permute_rows:
        Wx = Wx[PERM, :]
    return np.ascontiguousarray(Wx.astype(out_dt))


# --------------------------------------------------------------------------
# device kernel builder
# --------------------------------------------------------------------------

def _build(meta_sp, meta_te, ICOLS_sp, CCOLS_sp, ICOLS_te, CCOLS_te):
    ICOLS = max(ICOLS_sp, ICOLS_te)
    CCOLS = max(CCOLS_sp, CCOLS_te)
    S_MAX = max(m[0][4] + m[1][4] for m in (list(meta_sp) + list(meta_te)))
    ITMAX = max(
        (m[0][1] + m[1][1]) // 16 for m in (list(meta_sp) + list(meta_te))
    )

    nc = bacc.Bacc(
        "TRN2",
        target_bir_lowering=False,
        debug=False,
        num_devices=NCORES,
        dynamic_dma_scratch_size=SCRATCH,
    )
    f32, f16, i16 = dt.float32, dt.float16, dt.int16
    _mdt = {"f16": dt.float16, "f32": dt.float32, "bf16": dt.bfloat16}
    ztdt = _mdt[ZT_DT]

    def rdt_l(l):
        return dt.float32 if l <= max(NF32, NF32ROW) else _mdt[ROW_DT]

    def g2dt_l(l):
        return dt.float32 if l <= max(NF32, NF32G2) else _mdt[G2_DT]

    def row_l(l):
        return _row_elems(_lrow_dt(l))

    xT_t = nc.dram_tensor("xT", [P, SHARD], f32, kind="ExternalInput")
    idx_t, idxD_t, dfl_t = {}, {}, {}
    for g in ("sp", "te"):
        idx_t[g] = nc.dram_tensor(
            f"idx_{g}", [P, ICOLS + CCOLS * 8], i16, kind="ExternalInput"
        )
        dfl_t[g] = nc.dram_tensor(f"dfl_{g}", [P, CCOLS], f32, kind="ExternalInput")
    wext_t, gT_t, beT_t = {}, {}, {}
    for l in range(1, 5):
        kin = 128 if l == 1 else 256
        wext_t[l] = nc.dram_tensor(
            f"wext{l}", [kin, WCOL], f32 if l == 1 else ztdt, kind="ExternalInput"
        )
        gT_t[l] = nc.dram_tensor(f"gT{l}", [P, 2], f32, kind="ExternalInput")
        beT_t[l] = nc.dram_tensor(f"beT{l}", [P, 2], f32, kind="ExternalInput")
    wl_t = nc.dram_tensor("wl", [F, 32], ztdt, kind="ExternalInput")
    bl_t = nc.dram_tensor("bl2", [1, 32], ztdt, kind="ExternalInput")
    out_t = nc.dram_tensor("out", [SHARD, 32], f32, kind="ExternalOutput")
    simmode = bool(int(os.environ.get("GAT_SIMMODE", "0")))

    RG = [list(range(NCORES))]
    n_layers = int(os.environ.get("GAT_NLAYERS", "4"))

    with tile.TileContext(nc) as tc:
        with (
            tc.tile_pool(name="dram", bufs=1, space="DRAM") as dpool,
            tc.tile_pool(name="const", bufs=1) as cpool,
            tc.tile_pool(name="zpool", bufs=2) as zpool,
            tc.tile_pool(name="gidx", bufs=1) as gipool,
            tc.tile_pool(name="gpool", bufs=int(os.environ.get("GAT_GBUFS", "2"))) as gpool,
            tc.tile_pool(name="g2pool", bufs=2) as g2pool,
            tc.tile_pool(name="ohp", bufs=2) as ohpool,
            tc.tile_pool(name="small", bufs=6) as spool,
            tc.tile_pool(name="dgp", bufs=2) as dgpool,
            tc.tile_pool(name="itp", bufs=4) as itpool,
            tc.tile_pool(name="wpool", bufs=1) as wpool,
            tc.tile_pool(name="hpool", bufs=4) as hpool,
            tc.tile_pool(name="ypool", bufs=3) as ypool,
            tc.tile_pool(name="pbig", bufs=2, space="PSUM") as pbig,
            tc.tile_pool(name="pt", bufs=2, space="PSUM") as ptp,
            tc.tile_pool(name="ps", bufs=2, space="PSUM") as psp,
            tc.tile_pool(name="pstat", bufs=2, space="PSUM") as pstatp,
        ):
            # internal DRAM. Shared tensors allow a single writer -> per layer.
            ag_in = {
                dtn: [
                    dpool.tile(
                        [H1 if h else H0, _row_elems(dtn)],
                        dt.float32 if dtn == "f32" else dt.float16,
                        name=f"ag_in_{dtn}_{h}",
                    )
                    for h in range(2)
                ]
                for dtn in {_lrow_dt(l) for l in range(1, 5)}
            }
            ag_half = [
                [
                    dpool.tile(
                        [NCORES * (H1 if h else H0), row_l(l + 1)],
                        rdt_l(l + 1),
                        addr_space="Shared",
                        name=f"ag_h{l}_{h}",
                    )
                    for h in range(2)
                ]
                for l in range(4)
            ]
            aldDs = [
                dpool.tile([P, 64], f32, name=f"aldD{t}") for t in range(NTILE)
            ]
            y_dram = dpool.tile([SHARD, F], f32, name="y_dram")
            ar_in = dpool.tile([P, 4], f32, name="ar_in")
            ar_outs = [
                dpool.tile([P, 4], f32, addr_space="Shared", name=f"ar_out{l}")
                for l in range(4)
            ]

            def _scatter(out_ext, G2, dfl_sb, g2dt, l, t, S_t, ccur):
                if g2dt != f32:
                    # one batched is_equal builds all chunk one-hots:
                    # OH[p, c, j] = (iota[j] == dfl[p, c])
                    OH = ohpool.tile(
                        [P, S_MAX * P], g2dt, name=f"OH{l}_{t}", tag="OH"
                    )
                    OHv = OH[:].rearrange("p (s j) -> p s j", s=S_MAX)
                    iob = (
                        iotaF[:, 0:P]
                        .to_broadcast([P, P, S_t])
                        .transpose([0, 2, 1])
                    )
                    nc.vector.tensor_tensor(
                        out=OHv[:, 0:S_t, :],
                        in0=iob,
                        in1=dfl_sb[:, ccur : ccur + S_t].to_broadcast(
                            [P, S_t, P]
                        ),
                        op=ALU.is_equal,
                    )
                    for c in range(S_t):
                        nc.tensor.matmul(
                            out_ext[:],
                            OH[:, c * P : (c + 1) * P],
                            G2[:, c * 260 : (c + 1) * 260],
                            start=(c == 0),
                            stop=(c == S_t - 1),
                        )
                else:
                    for c in range(S_t):
                        onehot = spool.tile(
                            [P, P], g2dt, name=f"oh{l}_{t}_{c}", tag="oh"
                        )
                        nc.vector.tensor_scalar(
                            onehot[:],
                            iotaF[:],
                            dfl_sb[:, ccur + c : ccur + c + 1],
                            None,
                            ALU.is_equal,
                        )
                        nc.tensor.matmul(
                            out_ext[:],
                            onehot[:],
                            G2[:, c * 260 : (c + 1) * 260],
                            start=(c == 0),
                            stop=(c == S_t - 1),
                        )

            # ---- constants
            ident = cpool.tile([P, P], f32, name="ident")
            make_identity(nc, ident[:])
            iota32 = cpool.tile([P, P], dt.int32, name="iota32")
            nc.gpsimd.iota(iota32[:], pattern=[[1, P]], base=0, channel_multiplier=0)
            iotaF = cpool.tile([P, P], f32, name="iotaF")
            nc.vector.tensor_copy(iotaF[:], iota32[:])
            ones_col = cpool.tile([P, 1], f32, name="ones_col")
            nc.vector.memset(ones_col[:], 1.0)
            ones_row = cpool.tile([1, P], ztdt, name="ones_row")
            nc.vector.memset(ones_row[:], 1.0)

            zT = cpool.tile([P, 2 * SHARD], ztdt, name="zTbuf")
            nc.sync.dma_start(zT[:, 0:SHARD], xT_t[:])

            wl_sb = cpool.tile([P, 2, 32], ztdt, name="wl_sb")
            for k in range(2):
                nc.sync.dma_start(wl_sb[:, k, :], wl_t[k * P : (k + 1) * P, :])
            bl_sb = cpool.tile([1, 32], ztdt, name="bl_sb")
            nc.sync.dma_start(bl_sb[:], bl_t[:])

            PK = int(os.environ.get("GAT_PK", "3"))

            def _loads(l):
                graph = "sp" if l <= 2 else "te"
                wdt = f32 if l == 1 else ztdt
                d = {}
                if l in (1, 3):
                    dfl_sb = gipool.tile([P, CCOLS], f32, name=f"dfl{l}", tag="dfl")
                    nc.sync.dma_start(dfl_sb[:], dfl_t[graph][:])
                    d["dfl"] = dfl_sb
                wsb = wpool.tile([P, 2, WCOL], wdt, name=f"w{l}", tag="w")
                for k in range(1 if l == 1 else 2):
                    nc.sync.dma_start(wsb[:, k, :], wext_t[l][k * P : k * P + P, :])
                gT_sb = wpool.tile([P, 2], f32, name=f"g{l}", tag="gT")
                nc.sync.dma_start(gT_sb[:], gT_t[l][:])
                beT_sb = wpool.tile([P, 2], f32, name=f"be{l}", tag="beT")
                nc.sync.dma_start(beT_sb[:], beT_t[l][:])
                d["wsb"], d["gT"], d["beT"] = wsb, gT_sb, beT_sb
                return d

            def _mtile(l, t, wsb):
                cnt = _cnt(t)
                rdt = rdt_l(l)
                ROW = row_l(l)
                agi = ag_in[_lrow_dt(l)]
                KC = 1 if l == 1 else 2
                ph = pbig.tile([P, WCOL], f32, name=f"ph{l}_{t}", tag="pbig")
                for k in range(KC):
                    nc.tensor.matmul(
                        ph[:cnt, :],
                        zT[:, k * SHARD + t * P : k * SHARD + t * P + cnt],
                        wsb[:, k, :],
                        start=(k == 0),
                        stop=(k == KC - 1),
                    )
                hsb = hpool.tile([P, ROW], rdt, name=f"h{l}_{t}", tag="hsb")
                if rdt == dt.float32:
                    nc.scalar.activation(hsb[:cnt, :WCOL], ph[:cnt, :], AFT.Copy)
                else:
                    nc.scalar.activation(hsb[:cnt, :F], ph[:cnt, :F], AFT.Copy)
                    # als kept exact: f32 pair occupies f16 cols 256:264
                    nc.vector.tensor_copy(
                        hsb[:].bitcast(dt.float32)[:cnt, 128:132],
                        ph[:cnt, 256:260],
                    )
                aldf = hpool.tile([P, 4], f32, name=f"af{l}_{t}", tag="aldf")
                nc.vector.tensor_copy(aldf[:cnt, :], ph[:cnt, 260:264])
                r0, r1 = t * P, t * P + cnt
                if r1 <= H0:
                    nc.sync.dma_start(agi[0][r0:r1, :], hsb[:cnt, :])
                elif r0 >= H0:
                    nc.sync.dma_start(agi[1][r0 - H0 : r1 - H0, :], hsb[:cnt, :])
                else:
                    nc.sync.dma_start(agi[0][r0:H0, :], hsb[: H0 - r0, :])
                    nc.sync.dma_start(agi[1][0 : r1 - H0, :], hsb[H0 - r0 : cnt, :])
                nc.sync.dma_start(aldDs[t][:cnt, 0:4], aldf[:cnt, :])

            def _itile_ald(l, t):
                """Load the tile's combined idx table and issue its alD gathers
                (neither depends on the AllGather, so these can run early)."""
                meta = meta_sp if l <= 2 else meta_te
                graph = "sp" if l <= 2 else "te"
                S_t = meta[t][0][4] + meta[t][1][4]
                ccur = sum(m[0][4] + m[1][4] for m in meta[:t])
                ic0 = meta[t][0][0]
                itcols = (meta[t][0][1] + meta[t][1][1]) // 16
                tcols = itcols + S_t * 8
                itile = itpool.tile(
                    [P, ITMAX + S_MAX * 8], i16, name=f"it{l}_{t}", tag="it"
                )
                nc.sync.dma_start(
                    itile[:, :tcols],
                    idx_t[graph][:, ic0 + ccur * 8 : ic0 + ccur * 8 + tcols],
                )
                Dg = dgpool.tile([P, S_MAX * 64], f32, name=f"Dg{l}_{t}", tag="Dg")
                Dgv = Dg[:].rearrange("p (s r) -> p s r", s=S_MAX)
                nD = S_t * P
                offD = 0
                while offD < nD:
                    nn = min(GCAP, nD - offD)
                    nc.gpsimd.dma_gather(
                        Dgv[:, offD // P : (offD + nn) // P, :],
                        aldDs[t][:],
                        itile[:, itcols + offD // 16 : itcols + (offD + nn) // 16],
                        num_idxs=nn,
                        num_idxs_reg=nn,
                        elem_size=64,
                        elem_step=64,
                    )
                    offD += nn
                return itile, Dgv

            def _etile(l, t, dfl_sb, stats_sb, pre):
                cnt = _cnt(t)
                meta = meta_sp if l <= 2 else meta_te
                rdt = rdt_l(l)
                g2dt = g2dt_l(l)
                ROW = row_l(l)
                S_t = meta[t][0][4] + meta[t][1][4]
                ccur = sum(m[0][4] + m[1][4] for m in meta[:t])
                ic0 = meta[t][0][0]
                if t in pre:
                    itile, Dgv = pre.pop(t)
                else:
                    itile, Dgv = _itile_ald(l, t)
                G = gpool.tile([P, S_MAX * ROW], rdt, name=f"G{l}_{t}", tag="G")
                Gv = G[:].rearrange("p (s r) -> p s r", s=S_MAX)
                for ic, n, hh, cb, nchunk in meta[t]:
                    src_ap = ag_half[l - 1][hh][:]
                    off = 0
                    while off < n:
                        nn = min(GCAP, n - off)
                        icl = ic - ic0
                        nc.gpsimd.dma_gather(
                            Gv[:, cb + off // P : cb + (off + nn) // P, :],
                            src_ap,
                            itile[:, icl + off // 16 : icl + (off + nn) // 16],
                            num_idxs=nn,
                            num_idxs_reg=nn,
                            elem_size=ROW,
                            elem_step=ROW,
                        )
                        off += nn

                # batched per-tile edge scalars; als is f32 in the row
                if rdt == dt.float32:
                    alsv = Gv[:, 0:S_t, 256:260]
                else:
                    alsv = G[:].bitcast(dt.float32).rearrange(
                        "p (s r) -> p s r", s=S_MAX
                    )[:, 0:S_t, 128:132]
                tv = spool.tile([P, S_MAX * 4], f32, name=f"tv{l}_{t}", tag="tv")
                tvv = tv[:].rearrange("p (s r) -> p s r", s=S_MAX)
                nc.vector.tensor_tensor(
                    out=tvv[:, 0:S_t, :],
                    in0=alsv,
                    in1=Dgv[:, 0:S_t, 0:4],
                    op=ALU.add,
                )
                # clamp for f16-safe exp (softmax-invariant in practice)
                nc.vector.tensor_scalar(
                    tv[:, : S_t * 4], tv[:, : S_t * 4], 8.0, -15.0,
                    ALU.min, ALU.max,
                )
                # exp(leaky_relu(x)) = max(exp(x), exp(0.2x)); Exp stays in
                # one act-table set (no per-tile table reloads, unlike Lrelu)
                e1 = spool.tile([P, S_MAX * 4], f32, name=f"e1{l}_{t}", tag="e1")
                nc.scalar.activation(e1[:, : S_t * 4], tv[:, : S_t * 4], AFT.Exp)
                e2 = spool.tile([P, S_MAX * 4], f32, name=f"e2{l}_{t}", tag="e2")
                nc.scalar.activation(
                    e2[:, : S_t * 4], tv[:, : S_t * 4], AFT.Exp, scale=0.2
                )
                G2 = g2pool.tile(
                    [P, S_MAX * 260], g2dt, name=f"G2{l}_{t}", tag="G2"
                )
                G2v = G2[:].rearrange("p (s r) -> p s r", s=S_MAX)
                e1v = e1[:].rearrange("p (s r) -> p s r", s=S_MAX)
                e2v = e2[:].rearrange("p (s r) -> p s r", s=S_MAX)
                nc.vector.tensor_tensor(
                    out=G2v[:, 0:S_t, 256:260],
                    in0=e1v[:, 0:S_t, :],
                    in1=e2v[:, 0:S_t, :],
                    op=ALU.max,
                )
                # ex * h over the whole tile: packed 4-wide head dim (2x DVE)
                exb = (
                    G2v[:, 0:S_t, 256:260]
                    .to_broadcast([P, S_t, 4, C])
                    .transpose([0, 1, 3, 2])
                )
                nc.vector.tensor_tensor(
                    out=G2v[:, 0:S_t, 0:256].rearrange("p s (c h) -> p s c h", h=H),
                    in0=Gv[:, 0:S_t, 0:256].rearrange("p s (c h) -> p s c h", h=H),
                    in1=exb,
                    op=ALU.mult,
                )

                out_ext = pbig.tile([P, 260], f32, name=f"oe{l}_{t}", tag="pbig")
                _scatter(out_ext, G2, dfl_sb, g2dt, l, t, S_t, ccur)

                # tile epilogue: normalize, stats, store y
                rs = spool.tile([P, 4], f32, name=f"rs{l}_{t}", tag="rs")
                nc.vector.tensor_scalar(
                    rs[:], out_ext[:, 256:260], 1e-16, None, ALU.add
                )
                nc.vector.reciprocal(rs[:], rs[:])
                rsb = rs[:].to_broadcast([P, 4, C]).transpose([0, 2, 1])
                y_sb = ypool.tile([P, F], f32, name=f"y{l}_{t}", tag="y")
                nc.vector.tensor_tensor(
                    out=y_sb[:].rearrange("p (c h) -> p c h", h=H),
                    in0=out_ext[:, 0:F].rearrange("p (c h) -> p c h", h=H),
                    in1=rsb,
                    op=ALU.mult,
                )
                ysq = ypool.tile([P, F], f32, name=f"yq{l}_{t}", tag="ysq")
                nc.scalar.activation(ysq[:cnt, :], y_sb[:cnt, :], AFT.Square)
                stp = pstatp.tile([P, 4], f32, name=f"stp{l}_{t}", tag="pstat")
                for j, ssrc in enumerate(
                    (y_sb[:, 0:128], y_sb[:, 128:256], ysq[:, 0:128],
                     ysq[:, 128:256])
                ):
                    nc.tensor.matmul(
                        stp[:, j : j + 1],
                        ssrc[:cnt, :],
                        ones_col[:cnt, :],
                        start=True,
                        stop=True,
                    )
                nc.vector.tensor_tensor(
                    out=stats_sb[:], in0=stats_sb[:], in1=stp[:], op=ALU.add
                )
                nc.sync.dma_start(y_dram[t * P : t * P + cnt, :], y_sb[:cnt, :])

            def _bn_tile(l, t, Sb, Bb):
                cnt = _cnt(t)
                y2 = ypool.tile([P, F], f32, name=f"y2{l}_{t}", tag="y2")
                nc.sync.dma_start(y2[:cnt, :], y_dram[t * P : t * P + cnt, :])
                for k in range(2):
                    yTp = ptp.tile([P, P], f32, name=f"yt{l}_{t}_{k}", tag="pt")
                    nc.tensor.transpose(
                        yTp[:, :cnt],
                        y2[:cnt, k * P : (k + 1) * P],
                        ident[:cnt, :cnt],
                    )
                    nc.scalar.activation(
                        zT[:, k * SHARD + t * P : k * SHARD + t * P + cnt],
                        yTp[:, :cnt],
                        AFT.Relu,
                        bias=Bb[:, k : k + 1],
                        scale=Sb[:, k : k + 1],
                    )

            def _final_tile(t):
                cnt = _cnt(t)
                po = psp.tile([P, 32], f32, name=f"po{t}", tag="ps")
                for k in range(2):
                    nc.tensor.matmul(
                        po[:cnt, :],
                        zT[:, k * SHARD + t * P : k * SHARD + t * P + cnt],
                        wl_sb[:, k, :],
                        start=(k == 0),
                        stop=False,
                    )
                nc.tensor.matmul(
                    po[:cnt, :], ones_row[:, :cnt], bl_sb[:], start=False, stop=True
                )
                osb = hpool.tile([P, 32], f32, name=f"o{t}", tag="osb")
                nc.vector.tensor_copy(osb[:cnt, :], po[:cnt, :])
                nc.sync.dma_start(out_t[t * P : t * P + cnt, :], osb[:cnt, :])

            # ---- layer 1 M phase (standalone; layers 2-4 fuse M into the
            # previous layer's BN loop so the AllGather fires sooner)
            cur = _loads(1)
            dfl_cur = cur["dfl"]
            for t in range(NTILE):
                _mtile(1, t, cur["wsb"])
            pre = {t: _itile_ald(1, t) for t in range(PK)}

            for l in range(1, n_layers + 1):
                rdt = rdt_l(l)
                agi = ag_in[_lrow_dt(l)]
                for hh in range(2):
                    hn = H1 if hh else H0
                    if simmode:
                        nc.sync.dma_start(ag_half[l - 1][hh][0:hn, :], agi[hh][:])
                    else:
                        nc.gpsimd.collective_compute(
                            "AllGather",
                            ALU.bypass,
                            replica_groups=RG,
                            ins=[agi[hh][:]],
                            outs=[ag_half[l - 1][hh][:]],
                        )
                nxt = _loads(l + 1) if l < n_layers else None

                # ---- phase E
                stats_sb = wpool.tile([P, 4], f32, name=f"stats{l}", tag="stats")
                nc.vector.memset(stats_sb[:], 0.0)
                for t in range(NTILE):
                    _etile(l, t, dfl_cur, stats_sb, pre)
                pre = {}

                # ---- phase R
                nc.sync.dma_start(ar_in[:], stats_sb[:])
                ar_out = ar_outs[l - 1]
                if simmode:
                    nc.sync.dma_start(ar_out[:], ar_in[:])
                else:
                    nc.gpsimd.collective_compute(
                        "AllReduce",
                        ALU.add,
                        replica_groups=RG,
                        ins=[ar_in[:]],
                        outs=[ar_out[:]],
                    )
                arf = spool.tile([P, 4], f32, name=f"arf{l}", tag="arf")
                nc.sync.dma_start(arf[:], ar_out[:])
                mean = spool.tile([P, 2], f32, name=f"mean{l}", tag="mean")
                nc.vector.tensor_scalar(mean[:], arf[:, 0:2], 1.0 / N, None, ALU.mult)
                var = spool.tile([P, 2], f32, name=f"var{l}", tag="var")
                nc.vector.tensor_scalar(var[:], arf[:, 2:4], 1.0 / N, None, ALU.mult)
                msq = spool.tile([P, 2], f32, name=f"msq{l}", tag="msq")
                nc.vector.tensor_tensor(out=msq[:], in0=mean[:], in1=mean[:], op=ALU.mult)
                nc.vector.tensor_tensor(out=var[:], in0=var[:], in1=msq[:], op=ALU.subtract)
                nc.vector.tensor_scalar(var[:], var[:], EPS, None, ALU.add)
                sd = spool.tile([P, 2], f32, name=f"sd{l}", tag="sd")
                nc.scalar.activation(sd[:], var[:], AFT.Sqrt)
                nc.vector.reciprocal(sd[:], sd[:])
                Sb = wpool.tile([P, 2], f32, name=f"S{l}", tag="Sb")
                nc.vector.tensor_tensor(out=Sb[:], in0=cur["gT"][:], in1=sd[:], op=ALU.mult)
                Bb = wpool.tile([P, 2], f32, name=f"B{l}", tag="Bb")
                nc.vector.tensor_tensor(out=Bb[:], in0=mean[:], in1=Sb[:], op=ALU.mult)
                nc.vector.tensor_tensor(out=Bb[:], in0=cur["beT"][:], in1=Bb[:], op=ALU.subtract)

                # ---- fused BN(l) + M(l+1) (or final projection)
                for t in range(NTILE):
                    _bn_tile(l, t, Sb, Bb)
                    if l < n_layers:
                        _mtile(l + 1, t, nxt["wsb"])
                    else:
                        _final_tile(t)
                if l < n_layers:
                    if "dfl" in (nxt or {}):
                        dfl_cur = nxt["dfl"]
                    cur = nxt
                    pre = {t: _itile_ald(l + 1, t) for t in range(PK)}

    nc.compile()
    return nc


# --------------------------------------------------------------------------
# entry point
# --------------------------------------------------------------------------

def kernel(**inputs) -> np.ndarray:
    global LAST_RESULTS

    x = np.asarray(inputs["x"], np.float32)
    key = (
        int(np.asarray(inputs["edge_index_spatial"]).sum()),
        int(np.asarray(inputs["edge_index_temporal"]).sum()),
        "v4", ROW_DT, G2_DT, ZT_DT, NF32, NF32G2, NF32ROW,
    )
    if key in _CACHE:
        nc, packed = _CACHE[key]
    else:
        idx_sp, dfl_sp, meta_sp, IC_sp, CC_sp = _prep_graph(
            inputs["edge_index_spatial"]
        )
        idx_te, dfl_te, meta_te, IC_te, CC_te = _prep_graph(
            inputs["edge_index_temporal"]
        )
        ICOLS = max(IC_sp, IC_te)
        CCOLS = max(CC_sp, CC_te)
        packed = {
            "idx_sp": [_pad2(a, ICOLS + CCOLS * 8) for a in idx_sp],
            "idx_te": [_pad2(a, ICOLS + CCOLS * 8) for a in idx_te],
            "dfl_sp": [_pad2(a, CCOLS) for a in dfl_sp],
            "dfl_te": [_pad2(a, CCOLS) for a in dfl_te],
        }
        nc = _build(meta_sp, meta_te, IC_sp, CC_sp, IC_te, CC_te)
        _CACHE[key] = (nc, packed)

    in_maps = []
    for r in range(NCORES):
        m = {
            "xT": np.ascontiguousarray(x[r * SHARD : (r + 1) * SHARD, :].T),
            "wl": np.ascontiguousarray(
                np.asarray(inputs["Wl"], np.float32)[PERM, :].astype(_np_dt(ZT_DT))
            ),
            "bl2": np.ascontiguousarray(
                np.asarray(inputs["bl"], np.float32).reshape(1, 32).astype(_np_dt(ZT_DT))
            ),
        }
        for g in ("sp", "te"):
            m[f"idx_{g}"] = packed[f"idx_{g}"][r]
            m[f"dfl_{g}"] = packed[f"dfl_{g}"][r]
        for l in range(1, 5):
            m[f"wext{l}"] = _make_wext(
                np.asarray(inputs[f"W{l}"], np.float32),
                np.asarray(inputs[f"as{l}"], np.float32),
                np.asarray(inputs[f"ad{l}"], np.float32),
                permute_rows=(l > 1),
                out_dt=np.float32 if l == 1 else _np_dt(ZT_DT),
            )
            m[f"gT{l}"] = np.ascontiguousarray(
                np.asarray(inputs[f"g{l}"], np.float32)[PERM].reshape(2, P).T
            )
            m[f"beT{l}"] = np.ascontiguousarray(
                np.asarray(inputs[f"be{l}"], np.float32)[PERM].reshape(2, P).T
            )
        in_maps.append(m)

    trace = bool(int(os.environ.get("GAT_TRACE", "0")))
    try:
        res = bass_utils.run_bass_kernel_spmd(
            nc, in_maps, core_ids=list(range(NCORES)), trace=trace
        )
    except ModuleNotFoundError:
        res = bass_utils.run_bass_kernel_spmd(
            nc, in_maps, core_ids=list(range(NCORES)), trace=False
        )
    LAST_RESULTS = res
    return np.concatenate([res.results[r]["out"] for r in range(NCORES)], axis=0)


def _pad2(a, cols):
    if a.shape[1] == cols:
        return np.ascontiguousarray(a)
    out = np.zeros((a.shape[0], cols), a.dtype)
    out[:, : a.shape[1]] = a
    return out


def _np_dt(name):
    if name == "bf16":
        import ml_dtypes

        return ml_dtypes.bfloat16
    return {"f16": np.float16, "f32": np.float32}[name]
